# revision 6
# baseline (speedup 1.0000x reference)
"""MLA (multi-head latent attention) Trainium2 kernel, 8-core SPMD.

Sharding: pure head tensor-parallel — core c owns heads [4c, 4c+4) and ALL
2048 query rows.  Each core computes a full-shape partial output
pout_c = O(its 4 heads) @ w_out[rows of its heads]  in fp16; the host sums
the 8 partials and adds b_out.

Shared low-rank latents are computed cooperatively:
  phase B: C_KV + Kr for a 256-key slice per core  -> AllGather #1
  phase A: C_Q for (1024-row half x 3-of-12 m-chunks) per core -> AllGather #2
The 2D sharding of phase A keeps the per-core w_dq load at 3.15 MB.

All matmuls run in bf16 with fp32 PSUM accumulation.  Softmax skips
max-subtraction: logits are bounded (|S| < ~1.2 at this problem's scale).

Self-contained: shapes/layouts hardcoded; host does layout/cast/shard,
device kernel does all matmul/softmax work, host sums the 8 partials.
"""

import numpy as np
import ml_dtypes

import jax
from jax.sharding import Mesh, PartitionSpec, NamedSharding
try:
    from jax.experimental.shard_map import shard_map
except ImportError:  # newer jax
    from jax import shard_map

import concourse.tile as tile
from concourse import bacc, mybir
from concourse import bass2jax

BF16 = mybir.dt.bfloat16
F32 = mybir.dt.float32
F16 = mybir.dt.float16
AFT = mybir.ActivationFunctionType
ALU = mybir.AluOpType

# problem dims
S, DE, DC1, DC, DR, H, DH, DM = 2048, 4096, 1536, 512, 64, 32, 128, 4096
GH = 4                  # heads per core (8 cores x 4 heads = 32)
QB = 512                # query block width for attention inner loop
SCALER = float(1.0 / np.sqrt(np.float32(DH + DR)))
P = 128


def _emit_body(nc, tc, t):
    """One full iteration of the per-core computation.

    Phase order:
      B:    C_KVT/KrT for this core's 256-key slice -> AllGather #1 kickoff
      A:    C_QT for (row-half x 3 m-chunks)        -> AllGather #2 kickoff
      V/KT: V and K^T for my 4 heads (hides AG2 latency)
      Qproj: Q / Qr for my 4 heads over all 2048 rows
      C:    attention per (head, 512-query-block)
      D:    partial out-projection [2048, 4096]
    """
    from contextlib import ExitStack
    from concourse.tile_rust import add_dep_helper

    with ExitStack() as ctx:
        # PSUM pools: 2+3+1+2 = 8 banks exactly
        psg = ctx.enter_context(tc.tile_pool(name="psg", bufs=2, space="PSUM"))
        pss = ctx.enter_context(tc.tile_pool(name="pss", bufs=3, space="PSUM"))
        psd = ctx.enter_context(tc.tile_pool(name="psd", bufs=1, space="PSUM"))
        pso = ctx.enter_context(tc.tile_pool(name="pso", bufs=2, space="PSUM"))

        bpool = ctx.enter_context(tc.tile_pool(name="biases", bufs=1))
        pcw = ctx.enter_context(tc.tile_pool(name="pcw", bufs=2))
        iop_cm = tc.tile_pool(name="iop", bufs=1)
        iop = iop_cm.__enter__()
        pa_cm = tc.tile_pool(name="ph_a", bufs=1)
        pa = pa_cm.__enter__()

        # ---------- DMA ordering helper ----------
        crit_dmas = []

        def after_crit(bass_inst):
            for d in crit_dmas:
                add_dep_helper(bass_inst.ins, d,
                               reason="defer until B/A-critical DMAs issued")
            return bass_inst

        # ---------- B-critical loads (scalar queue) ----------
        wdkv_chunks = []
        for ch in range(4):
            wch = iop.tile([P, 8, DC], BF16, tag="wdkv", bufs=2, name=f"wdkv{ch}")
            ins = nc.scalar.dma_start(wch[:], t["wdkv"][:, ch * 8:(ch + 1) * 8, :])
            crit_dmas.append(ins.ins)
            wdkv_chunks.append(wch)
        wrk = iop.tile([P, 32, DR], BF16, tag="wrk", name="wrk")
        crit_dmas.append(nc.scalar.dma_start(wrk[:], t["wrk"][:]).ins)
        seqB = iop.tile([P, 32, 256], BF16, tag="seqB", name="seqB")
        for ch in range(4):
            ins = nc.sync.dma_start(seqB[:, ch * 8:(ch + 1) * 8, :],
                                    t["seqB"][:, ch * 8:(ch + 1) * 8, :])
            crit_dmas.append(ins.ins)

        # ---------- A loads (prefetch during B, sync queue) ----------
        seqA = pa.tile([P, 32, 1024], BF16, tag="seqA", name="seqA")
        for ch in range(8):
            nc.sync.dma_start(seqA[:, ch * 4:(ch + 1) * 4, :],
                              t["seqA"][:, ch * 4:(ch + 1) * 4, :])
        wdq_tiles = []
        for m in range(3):
            wq = pa.tile([P, 32, 128], BF16, tag="wdqq", bufs=2, name=f"wdqq{m}")
            nc.sync.dma_start(wq[:], t["wdq"][m])
            wdq_tiles.append(wq)

        # ---------- small persistent tiles (biases, ones) ----------
        ones128 = bpool.tile([P, P], BF16, tag="ones128", name="ones128")
        nc.any.memset(ones128[:], 1.0)
        bdq = bpool.tile([P, 3], F32, tag="bdq", name="bdq")
        bdkv = bpool.tile([P, 4], F32, tag="bdkv", name="bdkv")
        brk = bpool.tile([DR, 1], F32, tag="brk", name="brk")
        buq = bpool.tile([P, GH], F32, tag="buq", name="buq")
        brq = bpool.tile([DR, GH], F32, tag="brq", name="brq")
        buk = bpool.tile([P, GH], F32, tag="buk", name="buk")
        buv = bpool.tile([P, GH], F32, tag="buv", name="buv")
        for name, tl in [("bdq", bdq), ("bdkv", bdkv), ("brk", brk),
                         ("buq", buq), ("brq", brq), ("buk", buk),
                         ("buv", buv)]:
            nc.gpsimd.dma_start(tl[:], t[name][:])

        # ---------- attention weight streams (gpsimd queue, after crit) ----------
        wuv = pcw.tile([P, 4, 512], BF16, tag="wuv", name="wuv")
        after_crit(nc.gpsimd.dma_start(wuv[:], t["wuv"][:]))

        def load_wuk(h):
            w = pcw.tile([P, 4, DH], BF16, tag="wuk", name=f"wuk{h}")
            ins = nc.gpsimd.dma_start(w[:], t["wuk"][h])
            if h == 0:
                after_crit(ins)
            return w

        def load_wq(h):
            wuqh = pcw.tile([P, 12, DH], BF16, tag="wuq", name=f"wuq{h}")
            i1 = nc.gpsimd.dma_start(wuqh[:], t["wuq"][h])
            wrqh = pcw.tile([P, 12, DR], BF16, tag="wrq", name=f"wrq{h}")
            i2 = nc.gpsimd.dma_start(wrqh[:], t["wrq"][h])
            if h == 0:
                after_crit(i1)
                after_crit(i2)
            return wuqh, wrqh

        wuk_next = load_wuk(0)
        wq_next = load_wq(0)

        # ---------- Phase B (key-sharded) + AllGather #1 ----------
        pbd = ctx.enter_context(tc.tile_pool(name="ph_b_dram", bufs=1,
                                             space="DRAM"))
        with tc.tile_pool(name="ph_b", bufs=1) as pb:
            pack = pb.tile([P, 5, 256], BF16, tag="pack", name="pack")
            ps_m = [psg.tile([P, 256], F32, tag="psA", name=f"psB_{m}")
                    for m in range(2)] + \
                   [pss.tile([P, 256], F32, tag="s", name=f"psB_{m}")
                    for m in range(2, 4)]
            psk = pso.tile([DR, 256], F32, tag="o", name="psBk")
            for ch in range(4):
                for m in range(4):
                    for k8 in range(8):
                        ko = ch * 8 + k8
                        nc.tensor.matmul(ps_m[m][:],
                                         wdkv_chunks[ch][:, k8, m * P:(m + 1) * P],
                                         seqB[:, ko, :],
                                         start=(ko == 0), stop=(ko == 31))
                for k8 in range(8):
                    ko = ch * 8 + k8
                    nc.tensor.matmul(psk[:], wrk[:, ko, :], seqB[:, ko, :],
                                     start=(ko == 0), stop=(ko == 31))
            for m in range(4):
                nc.scalar.activation(pack[:, m, :], ps_m[m][:], AFT.Identity,
                                     bias=bdkv[:, m:m + 1])
            nc.scalar.activation(pack[0:DR, 4, :], psk[:], AFT.Identity,
                                 bias=brk[:, 0:1])
            gin = pbd.tile([P, 5, 256], BF16, tag="gin", name="gin")
            gout = pbd.tile([8, P, 5, 256], BF16, tag="gout", name="gout",
                            addr_space="Shared")
            nc.scalar.dma_start(gin[:], pack[:])
            nc.gpsimd.collective_compute(
                "AllGather",
                ALU.bypass,
                ins=[gin[:]],
                outs=[gout[:]],
                replica_groups=[list(range(8))],
            )

        # ---------- Phase A: C_QT chunk (row-half x 3 m-chunks) ----------
        with tc.tile_pool(name="ph_a_out", bufs=1) as pao:
            C_QTmy = pao.tile([P, 3, 1024], BF16, tag="C_QTmy", name="C_QTmy")
            for m in range(3):
                for half in range(2):
                    ps = psg.tile([P, QB], F32, tag="psA", name=f"psA{m}_{half}")
                    for ko in range(32):
                        nc.tensor.matmul(ps[:], wdq_tiles[m][:, ko, :],
                                         seqA[:, ko, half * QB:(half + 1) * QB],
                                         start=(ko == 0), stop=(ko == 31))
                    nc.scalar.activation(C_QTmy[:, m, half * QB:(half + 1) * QB],
                                         ps[:], AFT.Identity,
                                         bias=bdq[:, m:m + 1])
            gin2 = pbd.tile([P, 3, 1024], BF16, tag="gin2", name="gin2")
            gout2 = pbd.tile([8, P, 3, 1024], BF16, tag="gout2", name="gout2",
                             addr_space="Shared")
            nc.sync.dma_start(gin2[:], C_QTmy[:])
            nc.gpsimd.collective_compute(
                "AllGather",
                ALU.bypass,
                ins=[gin2[:]],
                outs=[gout2[:]],
                replica_groups=[list(range(8))],
            )
        pa_cm.__exit__(None, None, None)
        iop_cm.__exit__(None, None, None)

        # ---------- big persistent tiles (opened after A's pools close) ----------
        cpool = ctx.enter_context(tc.tile_pool(name="persist", bufs=1))
        cqt_cm = tc.tile_pool(name="cqt", bufs=1)
        cqtp = cqt_cm.__enter__()
        C_KVT = cpool.tile([P, 4, S], BF16, tag="C_KVT", name="C_KVT")
        KrT = cpool.tile([DR, S], BF16, tag="KrT", name="KrT")
        V_G = cpool.tile([P, 16, 512], BF16, tag="V_G", name="V_G")
        KT_all = cpool.tile([P, GH, S], BF16, tag="KT_all", name="KT_all")
        QTall = cpool.tile([P, GH, S], BF16, tag="qtall", name="QTall")
        QrTall = cpool.tile([DR, GH, S], BF16, tag="qrtall", name="QrTall")
        OT = cpool.tile([P, GH, S], BF16, tag="OT", name="OT")
        C_QT = cqtp.tile([P, 12, S], BF16, tag="C_QT", name="C_QT")

        # ---------- unpack AllGather #1 -> C_KVT, KrT ----------
        for m in range(4):
            nc.scalar.dma_start(
                C_KVT[:, m, :].rearrange("p (r n) -> p r n", r=8),
                gout[:, :, m, :].rearrange("r p n -> p r n"))
        nc.scalar.dma_start(
            KrT.rearrange("p (r n) -> p r n", r=8),
            gout[:, 0:DR, 4, :].rearrange("r p n -> p r n"))

        # ---------- V for my 4 heads (overlaps AG2) ----------
        for kt in range(16):
            ps = psg.tile([P, 512], F32, tag="psA", name=f"psV{kt}")
            for ci in range(4):
                nc.tensor.matmul(ps[:], C_KVT[:, ci, kt * P:(kt + 1) * P],
                                 wuv[:, ci, :],
                                 start=(ci == 0), stop=(ci == 3))
            nc.vector.tensor_copy(V_G[:, kt, :], ps[:])

        # ---------- K^T for my 4 heads (overlaps AG2) ----------
        for h in range(GH):
            wukh = wuk_next
            if h < GH - 1:
                wuk_next = load_wuk(h + 1)
            for kb in range(4):
                psk2 = psg.tile([P, 512], F32, tag="psA", name=f"psKT{h}_{kb}")
                for ci in range(4):
                    nc.tensor.matmul(psk2[:], wukh[:, ci, :],
                                     C_KVT[:, ci, kb * 512:(kb + 1) * 512],
                                     start=(ci == 0), stop=(ci == 3))
                nc.scalar.activation(KT_all[:, h, kb * 512:(kb + 1) * 512],
                                     psk2[:], AFT.Identity,
                                     bias=buk[:, h:h + 1])

        # ---------- unpack AllGather #2 -> C_QT ----------
        uengs = [nc.sync, nc.scalar]
        for r in range(8):
            for j in range(3):
                mm = 3 * (r % 4) + j
                roff = 1024 * (r // 4)
                uengs[(r * 3 + j) % 2].dma_start(
                    C_QT[:, mm, roff:roff + 1024], gout2[r, :, j, :])

        # ---------- Q / Qr projections for my 4 heads ----------
        for h in range(GH):
            wuqh, wrqh = wq_next
            if h < GH - 1:
                wq_next = load_wq(h + 1)
            for qb in range(4):
                ps = psg.tile([P, QB], F32, tag="psA", name=f"psQ{h}_{qb}")
                for ko in range(12):
                    nc.tensor.matmul(ps[:], wuqh[:, ko, :],
                                     C_QT[:, ko, qb * QB:(qb + 1) * QB],
                                     start=(ko == 0), stop=(ko == 11))
                nc.scalar.activation(QTall[:, h, qb * QB:(qb + 1) * QB],
                                     ps[:], AFT.Identity,
                                     bias=buq[:, h:h + 1], scale=SCALER)
                psr = psg.tile([DR, QB], F32, tag="psA", name=f"psQr{h}_{qb}")
                for ko in range(12):
                    nc.tensor.matmul(psr[:], wrqh[:, ko, :],
                                     C_QT[:, ko, qb * QB:(qb + 1) * QB],
                                     start=(ko == 0), stop=(ko == 11))
                nc.scalar.activation(QrTall[:, h, qb * QB:(qb + 1) * QB],
                                     psr[:], AFT.Identity,
                                     bias=brq[:, h:h + 1], scale=SCALER)
        cqt_cm.__exit__(None, None, None)

        # ---------- Phase D pool + first weight (prefetch during C) ----------
        pd = ctx.enter_context(tc.tile_pool(name="ph_d", bufs=1))
        wout_first = pd.tile([P, GH, 512], BF16, tag="wout", bufs=4,
                             name="wout0")
        nc.gpsimd.dma_start(wout_first[:], t["wout"][0])

        # ---------- Phase C: attention per (head, query-block) ----------
        with tc.tile_pool(name="ph_c", bufs=1) as pc:
            pending_den = []

            def flush_den(pd_item):
                hp, qbp, psO_p, psD_p = pd_item
                sl = slice(qbp * QB, (qbp + 1) * QB)
                recip = pc.tile([P, QB], F32, tag="recip", bufs=2,
                                name=f"recip{hp}_{qbp}")
                nc.vector.reciprocal(recip[:], psD_p[:])
                nc.vector.tensor_tensor(OT[:, hp, sl], psO_p[:], recip[:],
                                        ALU.mult)
                nc.scalar.activation(OT[:, hp, sl], OT[:, hp, sl], AFT.Identity,
                                     bias=buv[:, hp:hp + 1])

            for h in range(GH):
                for qb in range(4):
                    QT = QTall[:, h, qb * QB:(qb + 1) * QB]
                    QrT = QrTall[:, h, qb * QB:(qb + 1) * QB]
                    if pending_den:
                        flush_den(pending_den.pop(0))
                    PT = pc.tile([P, 16, QB], BF16, tag="pt", bufs=2,
                                 name=f"pt{h}_{qb}")
                    psO = pso.tile([P, QB], F32, tag="o", name=f"psO{h}_{qb}")
                    psD = psd.tile([P, QB], F32, tag="d", name=f"psD{h}_{qb}")
                    pending = None
                    for kt in range(16):
                        psS = pss.tile([P, QB], F32, tag="s",
                                       name=f"psS{h}_{qb}_{kt}")
                        nc.tensor.matmul(psS[:],
                                         KT_all[:, h, kt * P:(kt + 1) * P],
                                         QT, start=True, stop=False)
                        nc.tensor.matmul(psS[:], KrT[:, kt * P:(kt + 1) * P],
                                         QrT, start=False, stop=True)
                        nc.scalar.activation(PT[:, kt, :], psS[:], AFT.Exp)
                        if pending is not None:
                            kp = pending
                            nc.tensor.matmul(psO[:],
                                             V_G[:, kp, h * P:(h + 1) * P],
                                             PT[:, kp, :],
                                             start=(kp == 0), stop=False)
                            nc.tensor.matmul(psD[:], ones128[:], PT[:, kp, :],
                                             start=(kp == 0), stop=False)
                        pending = kt
                    kp = pending
                    nc.tensor.matmul(psO[:], V_G[:, kp, h * P:(h + 1) * P],
                                     PT[:, kp, :], start=False, stop=True)
                    nc.tensor.matmul(psD[:], ones128[:], PT[:, kp, :],
                                     start=False, stop=True)
                    pending_den.append((h, qb, psO, psD))
            for item in pending_den:
                flush_den(item)

        # ---------- Phase D: partial out-projection [2048, 4096] ----------
        for nt in range(8):
            if nt == 0:
                w = wout_first
            else:
                w = pd.tile([P, GH, 512], BF16, tag="wout", bufs=4,
                            name=f"wout{nt}")
                nc.gpsimd.dma_start(w[:], t["wout"][nt])
            for qt in range(16):
                ps = psg.tile([P, 512], F32, tag="psA", name=f"psOut{nt}_{qt}")
                for hh in range(GH):
                    nc.tensor.matmul(ps[:], OT[:, hh, qt * P:(qt + 1) * P],
                                     w[:, hh, :],
                                     start=(hh == 0), stop=(hh == GH - 1))
                osb = pd.tile([P, 512], F16, tag="osb", bufs=4,
                              name=f"osb{nt}_{qt}")
                nc.scalar.activation(osb[:], ps[:], AFT.Copy)
                oeng = nc.scalar if (nt * 16 + qt) % 2 == 0 else nc.gpsimd
                oeng.dma_start(
                    t["pout"][qt * P:(qt + 1) * P, nt * 512:(nt + 1) * 512],
                    osb[:])


def _build_program(rep=1):
    nc = bacc.Bacc("TRN2", target_bir_lowering=False, debug=False)
    t = {}
    t["seqA"] = nc.dram_tensor("t_seqA", [P, 32, 1024], BF16, kind="ExternalInput")
    t["seqB"] = nc.dram_tensor("t_seqB", [P, 32, 256], BF16, kind="ExternalInput")
    t["wdq"] = nc.dram_tensor("t_wdq", [3, P, 32, 128], BF16, kind="ExternalInput")
    t["wdkv"] = nc.dram_tensor("t_wdkv", [P, 32, DC], BF16, kind="ExternalInput")
    t["wrk"] = nc.dram_tensor("t_wrk", [P, 32, DR], BF16, kind="ExternalInput")
    t["wuq"] = nc.dram_tensor("t_wuq", [GH, P, 12, DH], BF16, kind="ExternalInput")
    t["wrq"] = nc.dram_tensor("t_wrq", [GH, P, 12, DR], BF16, kind="ExternalInput")
    t["wuk"] = nc.dram_tensor("t_wuk", [GH, P, 4, DH], BF16, kind="ExternalInput")
    t["wuv"] = nc.dram_tensor("t_wuv", [P, 4, 512], BF16, kind="ExternalInput")
    t["wout"] = nc.dram_tensor("t_wout", [8, P, GH, 512], BF16, kind="ExternalInput")
    t["bdq"] = nc.dram_tensor("t_bdq", [P, 3], F32, kind="ExternalInput")
    t["bdkv"] = nc.dram_tensor("t_bdkv", [P, 4], F32, kind="ExternalInput")
    t["brk"] = nc.dram_tensor("t_brk", [DR, 1], F32, kind="ExternalInput")
    t["buq"] = nc.dram_tensor("t_buq", [P, GH], F32, kind="ExternalInput")
    t["brq"] = nc.dram_tensor("t_brq", [DR, GH], F32, kind="ExternalInput")
    t["buk"] = nc.dram_tensor("t_buk", [P, GH], F32, kind="ExternalInput")
    t["buv"] = nc.dram_tensor("t_buv", [P, GH], F32, kind="ExternalInput")
    t["pout"] = nc.dram_tensor("t_pout", [S, DM], F16, kind="ExternalOutput")

    with tile.TileContext(nc) as tc:
        for _ in range(rep):
            _emit_body(nc, tc, t)
    nc.compile()
    return nc


def _prep_in_maps(inputs):
    """Host-side layout + bf16 cast -> per-core input dicts."""
    bf = ml_dtypes.bfloat16
    f32 = np.float32
    seq = np.asarray(inputs["sequence"], dtype=np.float32)[0]      # [2048, 4096]
    w_dq = np.asarray(inputs["w_dq"], dtype=np.float32)
    b_dq = np.asarray(inputs["b_dq"], dtype=np.float32)
    w_dkv = np.asarray(inputs["w_dkv"], dtype=np.float32)
    b_dkv = np.asarray(inputs["b_dkv"], dtype=np.float32)
    w_rk = np.asarray(inputs["w_rk"], dtype=np.float32)
    b_rk = np.asarray(inputs["b_rk"], dtype=np.float32)
    w_uq = np.asarray(inputs["w_uq"], dtype=np.float32)
    b_uq = np.asarray(inputs["b_uq"], dtype=np.float32)
    w_rq = np.asarray(inputs["w_rq"], dtype=np.float32)
    b_rq = np.asarray(inputs["b_rq"], dtype=np.float32)
    w_uk = np.asarray(inputs["w_uk"], dtype=np.float32)
    b_uk = np.asarray(inputs["b_uk"], dtype=np.float32)
    w_uv = np.asarray(inputs["w_uv"], dtype=np.float32)
    b_uv = np.asarray(inputs["b_uv"], dtype=np.float32)
    w_out = np.asarray(inputs["w_out"], dtype=np.float32)

    # seq halves transposed: [128, 32, 1024] per row-half
    seqT = [np.ascontiguousarray(
        seq[r * 1024:(r + 1) * 1024].reshape(1024, 32, P).transpose(2, 1, 0)
    ).astype(bf) for r in range(2)]
    wdq_all = w_dq.reshape(32, P, 12, 128).transpose(2, 1, 0, 3).astype(bf)
    bdq_all = np.ascontiguousarray(b_dq.reshape(12, P).T, dtype=f32)
    shared = {
        "wdkv": w_dkv.reshape(32, P, DC).transpose(1, 0, 2).astype(bf),
        "wrk": w_rk.reshape(32, P, DR).transpose(1, 0, 2).astype(bf),
        "bdkv": np.ascontiguousarray(b_dkv.reshape(4, P).T, dtype=f32),
        "brk": np.ascontiguousarray(b_rk.reshape(DR, 1), dtype=f32),
    }
    in_maps = []
    for c in range(8):
        r, mg = c // 4, c % 4
        cols = slice(c * GH * DH, (c + 1) * GH * DH)       # 512 model cols
        c1k = slice(c * GH * DR, (c + 1) * GH * DR)        # 256 rotary cols
        m = dict(shared)
        m["seqA"] = seqT[r]
        m["seqB"] = np.ascontiguousarray(seqT[r][:, :, 256 * mg:256 * (mg + 1)])
        m["wdq"] = np.ascontiguousarray(wdq_all[3 * mg:3 * (mg + 1)])
        m["bdq"] = np.ascontiguousarray(bdq_all[:, 3 * mg:3 * (mg + 1)])
        m["wuq"] = np.ascontiguousarray(
            w_uq[:, cols].reshape(12, P, GH, DH).transpose(2, 1, 0, 3)).astype(bf)
        m["wrq"] = np.ascontiguousarray(
            w_rq[:, c1k].reshape(12, P, GH, DR).transpose(2, 1, 0, 3)).astype(bf)
        m["wuk"] = np.ascontiguousarray(
            w_uk[:, cols].reshape(4, P, GH, DH).transpose(2, 1, 0, 3)).astype(bf)
        m["wuv"] = np.ascontiguousarray(
            w_uv[:, cols].reshape(4, P, 512).transpose(1, 0, 2)).astype(bf)
        m["wout"] = np.ascontiguousarray(
            w_out[cols, :].reshape(GH, P, 8, 512).transpose(2, 1, 0, 3)).astype(bf)
        m["buq"] = np.ascontiguousarray((b_uq[cols] * SCALER).reshape(GH, P).T,
                                        dtype=f32)
        m["brq"] = np.ascontiguousarray((b_rq[c1k] * SCALER).reshape(GH, DR).T,
                                        dtype=f32)
        m["buk"] = np.ascontiguousarray(b_uk[cols].reshape(GH, P).T, dtype=f32)
        m["buv"] = np.ascontiguousarray(b_uv[cols].reshape(GH, P).T, dtype=f32)
        in_maps.append({f"t_{k}": v for k, v in m.items()})
    return in_maps


class _Runner:
    """Cached sharded PJRT executor for a compiled Bass program."""

    def __init__(self, nc):
        bass2jax.install_neuronx_cc_hook()
        self.nc = nc
        in_names, out_names, out_avals = [], [], []
        pid_name = nc.partition_id_tensor.name if nc.partition_id_tensor else None
        for alloc in nc.m.functions[0].allocations:
            if not isinstance(alloc, mybir.MemoryLocationSet):
                continue
            name = alloc.memorylocations[0].name
            if alloc.kind == "ExternalInput":
                if name != pid_name:
                    in_names.append(name)
            elif alloc.kind == "ExternalOutput":
                out_names.append(name)
                shape = tuple(alloc.tensor_shape)
                dtype = mybir.dt.np(alloc.dtype)
                out_avals.append(jax.core.ShapedArray(shape, dtype))
        self.in_names = in_names
        self.out_names = out_names
        all_in_names = list(in_names) + list(out_names)
        if pid_name is not None:
            all_in_names.append(pid_name)

        def _body(*args):
            operands = list(args)
            if nc.partition_id_tensor is not None:
                operands.append(bass2jax.partition_id_tensor())
            outs = bass2jax._bass_exec_p.bind(
                *operands,
                out_avals=tuple(out_avals),
                in_names=tuple(all_in_names),
                out_names=tuple(out_names),
                lowering_input_output_aliases=(),
                sim_require_finite=True,
                sim_require_nnan=True,
                nc=nc,
            )
            return tuple(outs)

        devices = jax.devices()[:8]
        self.mesh = Mesh(np.asarray(devices), ("core",))
        n_io = len(in_names) + len(out_names)
        self.fn = jax.jit(
            shard_map(_body, mesh=self.mesh,
                      in_specs=(PartitionSpec("core"),) * n_io,
                      out_specs=(PartitionSpec("core"),) * len(out_names),
                      check_rep=False),
            keep_unused=True)
        self.sharding = NamedSharding(self.mesh, PartitionSpec("core"))
        self.dev_zero = [
            jax.device_put(
                np.zeros((8 * av.shape[0], *av.shape[1:]), av.dtype), self.sharding)
            for av in out_avals]
        self.out_avals = out_avals

    def stage(self, in_maps):
        dev_in = []
        for name in self.in_names:
            cat = np.concatenate([np.asarray(m[name]) for m in in_maps], axis=0)
            dev_in.append(jax.device_put(cat, self.sharding))
        return dev_in

    def run_staged(self, dev_in):
        outs = self.fn(*dev_in, *self.dev_zero)
        jax.block_until_ready(outs)
        return outs

    def run(self, in_maps):
        outs = self.run_staged(self.stage(in_maps))
        res = []
        for c in range(8):
            d = {}
            for i, name in enumerate(self.out_names):
                av = self.out_avals[i]
                d[name] = np.asarray(outs[i]).reshape(8, *av.shape)[c]
            res.append(d)
        return res


_CTX = None


def _get_ctx():
    global _CTX
    if _CTX is None:
        nc = _build_program(rep=1)
        _CTX = _Runner(nc)
    return _CTX


def kernel(**inputs):
    runner = _get_ctx()
    in_maps = _prep_in_maps(inputs)
    res = runner.run(in_maps)
    b_out = np.asarray(inputs["b_out"], dtype=np.float32)
    acc = res[0]["t_pout"].astype(np.float32)
    for c in range(1, 8):
        acc += res[c]["t_pout"].astype(np.float32)
    return (acc + b_out).reshape(1, S, DM)
